# revision 45
# baseline (speedup 1.0000x reference)
"""Grouped MoE MLP (SwiGLU) for Trainium2, expert-parallel across 8 NeuronCores.

Problem: out = gmm(silu(gmm(x,Wg)) * gmm(x,Wu), Wd) with E=8 experts,
T=8192 tokens pre-sorted by expert, H=2048, I=4096.

Strategy: expert parallelism — core e computes expert e's tokens end-to-end.
The host splits the (ragged) token dim by expert, pads each group to a fixed
capacity C, casts to bf16, and PRE-TILES every tensor into the exact SBUF
layout the device consumes, so every DMA is a plain contiguous 2D copy at
full line rate (2-8 KB lines, minimal descriptors, minimal issue cost).

Device program per core (all shapes hardcoded at build time):
  GEMM1 computes the SwiGLU intermediate TRANSPOSED (interT[I, C]) so that
  GEMM2's contraction dim (I) is already the partition dim — no on-device
  transposes anywhere. bf16 inputs, fp32 PSUM accumulation, bf16 output.

Ramp engineering (the steady-state MM stream is already at the 216 ns/MM
N=512 floor, so the wins are at the edges):
  * ~14 warmup matmuls on a zeroed scratch tile run during the DMA ramp so
    the PE_HAM clock gate reaches K=8/8 (2.4 GHz) before real work arrives.
  * The first weight chunk is one k-slice (64 KB) and the first xt slice is
    one (k, t)-quarter (128 KB), so the first real matmul fires ~2 us
    earlier than with coarse transfers.
  * All t=0 halves of xt stream before t=1 halves, alternating rings by k
    parity, and pair-0's psum groups run t-outer — the PE consumes data in
    exactly the order it lands.
"""

import numpy as np
import ml_dtypes

P = 128          # partition dim
NB = 512         # matmul moving free dim / PSUM bank width (fp32)
E, T, H, I = 8, 8192, 2048, 4096
C_DEFAULT = T // E  # per-expert token capacity
WARMUP = 28      # HAM warmup matmuls (std build, N=128)
WARMUP_ST = 12   # strassen build: N=512 warmups, ~4 us of PE filler
STRASSEN = True  # one-level Strassen-Winograd on the gate/up GEMMs

_NC_CACHE = {}


def _build(C, Hd, Id, nb=NB):
    """Build + bacc-compile the per-core Tile program. Returns the Bass module."""
    import concourse.bass as bass  # noqa: F401
    import concourse.tile as tile
    from concourse import bacc, mybir

    bf16 = mybir.dt.bfloat16
    f32 = mybir.dt.float32
    KT = Hd // P       # GEMM1 contraction tiles (over H)
    IT = Id // P       # i-tiles (GEMM1 output partitions / GEMM2 contraction)
    TT = C // nb       # token blocks for GEMM1 moving operand
    T8 = C // P        # token tiles for GEMM2 output partitions
    HB = Hd // nb      # h-blocks for GEMM2 moving operand
    IP = IT // 2       # i-block pairs
    W2 = 2 * P         # pair width in I columns

    nc = bacc.Bacc(
        "TRN2",
        target_bir_lowering=False,
        debug=False,
        enable_asserts=False,
        num_devices=8,
    )
    # Host-pre-tiled layouts (see _prepare):
    #   xT[p, k*C + c]   = x[c, k*P + p]
    #   wg[pp*P+p, k*W2+c] = gate[k*P+p, pp*W2+c]   (same for wu)
    #   wd[h*P+p, k*nb+c]  = down[k*P+p, h*nb+c]
    xT = nc.dram_tensor("xT", [P, KT * C], bf16, kind="ExternalInput").ap()
    wg = nc.dram_tensor("wg", [IP * P, KT * W2], bf16, kind="ExternalInput").ap()
    wu = nc.dram_tensor("wu", [IP * P, KT * W2], bf16, kind="ExternalInput").ap()
    wd = nc.dram_tensor("wd", [HB * P, IT * nb], bf16, kind="ExternalInput").ap()
    out = nc.dram_tensor("out", [C, Hd], bf16, kind="ExternalOutput").ap()

    def k3(ap):
        return ap.rearrange("p (k c) -> p k c", c=C)

    with tile.TileContext(nc) as tc:
        with tc.tile_pool(name="res", bufs=1) as res:
            # SwiGLU intermediate, transposed: interT[p, i*C + c] = inter[c, i*P+p]
            interT = res.tile([P, IT * C], bf16)
            # h=0 block of Wd, prefetched during phase 1 so phase 2 starts hot
            wd0 = res.tile([P, IT * nb], bf16)

            # ps1 spans BOTH phases (6 banks) so the allocator must give ps2
            # disjoint fresh banks — otherwise the first phase-2 matmul
            # inherits a conservative wait on ALL phase-1 matmuls completing.
            with tc.tile_pool(name="ps1", bufs=2, space="PSUM") as ps1:
              # ------------- Phase 1: gate/up GEMMs + SwiGLU -------------
              with tc.tile_pool(name="wrm", bufs=1, space="PSUM") as wrm, \
                 tc.tile_pool(name="p1x", bufs=1) as p1x, \
                 tc.tile_pool(name="w1", bufs=3) as w1, \
                 tc.tile_pool(name="tmp1", bufs=4) as tmp1:
                  # HAM warmup: keep the PE busy during the DMA ramp so the
                  # clock gate opens to 2.4 GHz before the first real matmul.
                  scr = p1x.tile([P, P], bf16)
                  nc.vector.memset(scr[:], 0.0)
                  psw = wrm.tile([P, P], f32)
                  for _ in range(WARMUP):
                      nc.tensor.matmul(psw[:], scr[:], scr[:],
                                       start=True, stop=True)

                  # Baseline-proven ramp: few, large DMAs (each DMA carries
                  # ~1-2 us of fixed queue overhead, so fine-grained ramps
                  # lose).  xt loads as KT 256 KB k-slices alternating
                  # rings; pair-0 weights load j0-halves first in two
                  # k-chunks, j1-halves after the ramp-critical stream.
                  wgp0 = w1.tile([P, KT * W2], bf16, tag="wg")
                  wup0 = w1.tile([P, KT * W2], bf16, tag="wu")
                  xt = p1x.tile([P, KT * C], bf16)
                  KH = KT // 2

                  def j3(tl):
                      return tl.rearrange("p (k c) -> p k c", c=W2)

                  wgv = j3(wg[0:P, :])
                  wuv = j3(wu[0:P, :])
                  g0v = j3(wgp0[:, :])
                  u0v = j3(wup0[:, :])
                  for k in range(KT):
                      eng = nc.sync if k % 2 == 0 else nc.scalar
                      eng.dma_start(xt[:, k * C:(k + 1) * C],
                                    xT[:, k * C:(k + 1) * C])
                      if k == 0:
                          nc.sync.dma_start(g0v[:, 0:KH, 0:P], wgv[:, 0:KH, 0:P])
                          nc.scalar.dma_start(u0v[:, 0:KH, 0:P], wuv[:, 0:KH, 0:P])
                      elif k == 2:
                          nc.sync.dma_start(g0v[:, KH:KT, 0:P], wgv[:, KH:KT, 0:P])
                          nc.scalar.dma_start(u0v[:, KH:KT, 0:P], wuv[:, KH:KT, 0:P])
                  # j=1 columns of pair 0, after the ramp-critical stream
                  nc.sync.dma_start(g0v[:, :, P:W2], wgv[:, :, P:W2])
                  nc.scalar.dma_start(u0v[:, :, P:W2], wuv[:, :, P:W2])

                  for p in range(IP):
                    if p == 0:
                        wgp, wup = wgp0, wup0
                    else:
                        wgp = w1.tile([P, KT * W2], bf16, tag="wg")
                        wup = w1.tile([P, KT * W2], bf16, tag="wu")
                        for k0, k1 in ((0, KH), (KH, KT)):
                            nc.sync.dma_start(wgp[:, k0 * W2:k1 * W2],
                                              wg[p * P:(p + 1) * P, k0 * W2:k1 * W2])
                            nc.scalar.dma_start(wup[:, k0 * W2:k1 * W2],
                                                wu[p * P:(p + 1) * P, k0 * W2:k1 * W2])
                        if p == 4:
                            # prefetch Wd h=0 once the startup ramp has
                            # drained; phase 2 needs it at ~2/3 of the span
                            for d in range(2):
                                kk = IT // 2
                                eng = nc.sync if d % 2 == 0 else nc.scalar
                                eng.dma_start(
                                    wd0[:, d * kk * nb:(d + 1) * kk * nb],
                                    wd[0:P, d * kk * nb:(d + 1) * kk * nb])
                    jts = [(j, t) for j in range(2) for t in range(TT)]
                    for j, t in jts:
                        i = 2 * p + j
                        psg = ps1.tile([P, nb], f32, tag=f"g{t}")
                        psu = ps1.tile([P, nb], f32, tag=f"u{t}", bufs=1)
                        for k in range(KT):
                            rhs = xt[:, k * C + t * nb: k * C + t * nb + nb]
                            lhs = wgp[:, k * W2 + j * P: k * W2 + (j + 1) * P]
                            nc.tensor.matmul(psg[:], lhs, rhs,
                                             start=(k == 0), stop=(k == KT - 1))
                        for k in range(KT):
                            rhs = xt[:, k * C + t * nb: k * C + t * nb + nb]
                            lhs = wup[:, k * W2 + j * P: k * W2 + (j + 1) * P]
                            nc.tensor.matmul(psu[:], lhs, rhs,
                                             start=(k == 0), stop=(k == KT - 1))
                        # silu(g)*u = sigmoid(g)*g*u; each DVE op may
                        # read at most ONE operand from PSUM.
                        sig = tmp1.tile([P, nb], f32, tag="sig")
                        nc.scalar.activation(
                            sig[:], psg[:], mybir.ActivationFunctionType.Sigmoid)
                        sg = tmp1.tile([P, nb], f32, tag="sg")
                        nc.vector.tensor_mul(sg[:], sig[:], psg[:])
                        nc.vector.tensor_mul(
                            interT[:, i * C + t * nb: i * C + t * nb + nb],
                            sg[:], psu[:])

              # ---------------- Phase 2: down GEMM ----------------
              with tc.tile_pool(name="w2", bufs=2) as w2, \
                 tc.tile_pool(name="ps2", bufs=2, space="PSUM") as ps2, \
                 tc.tile_pool(name="ot2", bufs=4) as ot2:
                  for h in range(HB):
                    if h == 0:
                        wdh = wd0
                    else:
                        wdh = w2.tile([P, IT * nb], bf16, tag="wd")
                        for d in range(2):
                            kk = IT // 2
                            eng = nc.sync if d % 2 == 0 else nc.scalar
                            eng.dma_start(
                                wdh[:, d * kk * nb:(d + 1) * kk * nb],
                                wd[h * P:(h + 1) * P,
                                   d * kk * nb:(d + 1) * kk * nb])
                    for t in range(T8):
                        ps = ps2.tile([P, nb], f32, tag="o")
                        for k in range(IT):
                            nc.tensor.matmul(
                                ps[:],
                                interT[:, k * C + t * P: k * C + t * P + P],
                                wdh[:, k * nb:(k + 1) * nb],
                                start=(k == 0), stop=(k == IT - 1))
                        ot = ot2.tile([P, nb], bf16, tag="ot")
                        nc.scalar.copy(ot[:], ps[:])
                        nc.sync.dma_start(out[t * P:(t + 1) * P, h * nb:(h + 1) * nb], ot[:])

    nc.compile()
    return nc


def _use_strassen(C, Hd, Id):
    return (STRASSEN and C == 2 * NB and Hd % (2 * P) == 0
            and Id % (2 * P) == 0)


def _build_strassen(C, Hd, Id, nb=NB):
    """One-level Strassen-Winograd on the gate/up GEMMs (7/8 of the matmul
    passes).  Both operand sides of GEMM1 are kernel inputs, so all 7
    operand combinations are formed on the host for free; the device only
    pays the output-side combination adds, which ride the mostly-idle
    vector engine.  C11 = P1 + P2 is accumulated directly in PSUM (P1's
    chain is left open, P1 is copied out for U1, then P2's chain continues
    into the same bank)."""
    import concourse.bass as bass  # noqa: F401
    import concourse.tile as tile
    from concourse import bacc, mybir

    bf16 = mybir.dt.bfloat16
    f32 = mybir.dt.float32
    kt = Hd // (2 * P)     # contraction tiles per product (Kh/P = 8)
    mt = Id // (2 * P)     # m-tiles per M-half (16)
    IT = Id // P
    T8 = C // P
    HB = Hd // nb
    assert C == 2 * nb

    nc = bacc.Bacc(
        "TRN2",
        target_bir_lowering=False,
        debug=False,
        enable_asserts=False,
        num_devices=8,
    )
    # Host-pre-tiled (see _prepare_strassen_expert):
    #   xs[q*P+p, k*nb+c]        = Bq[k*P+p, c]        (7 x-combos, [Kh, Nh])
    #   wgs[(q*mt+m)*P+p, k*P+c] = Gq[k*P+p, m*P+c]    (7 w-combos, [Kh, Mh])
    #   wd[h*P+p, k*nb+c]        = down[k*P+p, h*nb+c]
    kt2 = Id // (2 * P)    # phase-2 contraction tiles per product (16)
    mtH = Hd // (2 * P)    # phase-2 m-tiles per M-half (8)
    KT = 2 * kt
    # x ships as plain pre-tiled quadrants; the 4 T-combos are built by the
    # otherwise-idle DVE during the DMA ramp (3 MB less ramp traffic)
    xT = nc.dram_tensor("xT", [P, KT * C], bf16, kind="ExternalInput").ap()
    wgs = nc.dram_tensor("wgs", [7 * mt * P, kt * P], bf16,
                         kind="ExternalInput").ap()
    wus = nc.dram_tensor("wus", [7 * mt * P, kt * P], bf16,
                         kind="ExternalInput").ap()
    wds = nc.dram_tensor("wds", [7 * mtH * P, kt2 * P], bf16,
                         kind="ExternalInput").ap()
    # output is transposed ([Hd, C]); the host untransposes for free
    out = nc.dram_tensor("out", [Hd, C], bf16, kind="ExternalOutput").ap()

    ORDER = [0, 5, 4, 2, 6, 3, 1]   # P1, P6, P5, P3, P7, P4, then P2
    ORDER_M0 = [0, 4, 2, 6, 1, 5, 3]  # position 0: P1 P5 P3 P7 P2 P6 P4
    BTAG = {5: "b", 4: "c", 2: "d", 6: "e", 3: "f"}
    QORD2 = [0, 1, 2, 4, 6, 5, 3]   # phase 2: P1 P2 P3 P5 P7 P6 P4'
    Sig = mybir.ActivationFunctionType.Sigmoid

    with tile.TileContext(nc) as tc:
        with tc.tile_pool(name="res", bufs=1) as res, \
             tc.tile_pool(name="w2r", bufs=4) as w2r:
            interT = res.tile([P, IT * C], bf16)
            xt = None   # x quadrants, allocated inside the phase-1 scope
            xcr = None  # T-combos (T4n | T1 | T2 | T3), ditto

            def rhs_q(q, k):
                if q == 0:   # B11
                    return xt[:, k * C: k * C + nb]
                if q == 1:   # B21
                    return xt[:, (kt + k) * C: (kt + k) * C + nb]
                if q == 2:   # B22
                    return xt[:, (kt + k) * C + nb: (kt + k + 1) * C]
                slot = {3: 0, 4: 1, 5: 2, 6: 3}[q]
                return xcr[:, (slot * kt + k) * nb: (slot * kt + k + 1) * nb]

            def b12(k):
                return xt[:, k * C + nb: (k + 1) * C]

            def wsrc(ws, q, m, k0, k1):
                return ws[(q * mt + m) * P:(q * mt + m + 1) * P, k0 * P:k1 * P]

            def stat_load(q2, m, eng):
                st_ = w2r.tile([P, kt2 * P], bf16, tag="st")
                eng.dma_start(st_[:, :],
                              wds[(q2 * mtH + m) * P:(q2 * mtH + m + 1) * P, :])
                return st_

            stat0 = []  # pass-0 stationary tiles, prefetched in phase-1 tail

            with tc.tile_pool(name="psS", bufs=1, space="PSUM") as psS:
              with tc.tile_pool(name="wa", bufs=12) as wa, \
                 tc.tile_pool(name="p1x", bufs=1) as p1x, \
                 tc.tile_pool(name="tmp", bufs=1) as tmp:
                  xt = p1x.tile([P, KT * C], bf16)
                  xcr = p1x.tile([P, 4 * kt * nb], bf16)
                  scr = tmp.tile([P, nb], bf16, tag="scr")
                  nc.vector.memset(scr[:], 0.0)
                  psw = psS.tile([P, nb], f32, tag="a", bufs=2)
                  for _ in range(WARMUP_ST):
                      nc.tensor.matmul(psw[:], scr[:, 0:P], scr[:],
                                       start=True, stop=True)

                  def load_wa(ws, m, eng0):
                      tiles = {}
                      for qi, q in enumerate(ORDER):
                          t_ = wa.tile([P, kt * P], bf16, tag="wa")
                          eng = nc.sync if (qi + eng0) % 2 == 0 else nc.scalar
                          eng.dma_start(t_[:, :], wsrc(ws, q, m, 0, kt))
                          tiles[q] = t_
                      return tiles

                  # ---- ramp: x quadrants + m0 stationary tiles in
                  # consumption order, few large DMAs (each DMA costs ~1-2
                  # us of fixed queue overhead, so fine-grained ramps lose).
                  # The 4 T-combos are built by the DVE as quadrants land. ----
                  kh = kt // 2
                  wam_g, wam_u = {}, {}
                  ring = [nc.sync, nc.scalar]
                  ri = 0

                  def nxt():
                      nonlocal ri
                      ri += 1
                      return ring[ri % 2]

                  def wload(ws, wam, q):
                      t_ = wa.tile([P, kt * P], bf16, tag="wa")
                      nxt().dma_start(t_[:, :], wsrc(ws, q, 0, 0, kt))
                      wam[q] = t_

                  xtv = xt[:, :].rearrange("p (k c) -> p k c", c=C)
                  xTv = xT[:, :].rearrange("p (k c) -> p k c", c=C)

                  def xquad2(krow, chalf):
                      o = kt if krow else 0
                      c0 = nb if chalf else 0
                      for eng, (k0, k1) in zip((nc.sync, nc.scalar),
                                               ((0, kh), (kh, kt))):
                          sl = (slice(None), slice(o + k0, o + k1),
                                slice(c0, c0 + nb))
                          eng.dma_start(xtv[sl], xTv[sl])

                  def cs(slot, k):
                      return xcr[:, (slot * kt + k) * nb:
                                 (slot * kt + k + 1) * nb]

                  # m0 consumption order: P1 P5 P3 P7 P2 P6 P4 (products on
                  # device-built T-combos last, since the DVE chain that
                  # builds them is the ramp's critical path)
                  wload(wgs, wam_g, 0)    # P1
                  xquad2(0, 0)            # B11
                  xquad2(0, 1)            # B12
                  xquad2(1, 1)            # B22
                  xquad2(1, 0)            # B21
                  # T-combos on the DVE, dependency order
                  for k in range(kt):     # T1 = B12 - B11
                      nc.vector.tensor_sub(cs(1, k), b12(k), rhs_q(0, k))
                  for k in range(kt):     # T3 = B22 - B12
                      nc.vector.tensor_sub(cs(3, k), rhs_q(2, k), b12(k))
                  for k in range(kt):     # T2 = T3 + B11
                      nc.vector.tensor_add(cs(2, k), cs(3, k), rhs_q(0, k))
                  for k in range(kt):     # T4n = B21 - T2
                      nc.vector.tensor_sub(cs(0, k), rhs_q(1, k), cs(2, k))
                  for q in [4, 2, 6, 1, 5, 3]:      # gate P5 P3 P7 P2 P6 P4
                      wload(wgs, wam_g, q)
                  for q in ORDER_M0:                # up, same order
                      wload(wus, wam_u, q)

                  st = {}  # cross-half-position state (gate C-tiles, sg11)

                  def do_product(q, wam, S, is_up, m):
                      """One 8-MM product pass + its drain ops.  S carries
                      the per-weight running tiles (p1/u1/v/w/psA/c*)."""
                      if q in (0, 1):
                          if q == 0:
                              S["psA"] = psS.tile([P, nb], f32, tag="a",
                                                  bufs=2, name=f"psA_{m}")
                          pt = S["psA"]
                      else:
                          pt = psS.tile([P, nb], f32, tag=BTAG[q], bufs=1,
                                        name=f"ps_{BTAG[q]}_{m}")
                      first = (q != 1)
                      for k in range(kt):
                          nc.tensor.matmul(
                              pt[:], wam[q][:, k * P:(k + 1) * P],
                              rhs_q(q, k),
                              start=(first and k == 0),
                              stop=((not first) and k == kt - 1),
                              skip_group_check=True)
                      if q == 0:
                          p1 = tmp.tile([P, nb], bf16, tag="p1", bufs=2)
                          nc.scalar.copy(p1[:], pt[:])
                          S["p1"] = p1
                      elif q == 5:
                          u1 = tmp.tile([P, nb], bf16, tag="u1", bufs=2)
                          nc.vector.tensor_add(u1[:], S["p1"][:], pt[:])
                          S["u1"] = u1
                      elif q == 4:
                          v = tmp.tile([P, nb], bf16, tag="v", bufs=2)
                          nc.vector.tensor_add(v[:], S["u1"][:], pt[:])
                          S["v"] = v
                      elif q == 2:      # C12 = V + P3
                          c12 = tmp.tile([P, nb], bf16, tag="c12g", bufs=2)
                          nc.vector.tensor_add(c12[:], S["v"][:], pt[:])
                          if is_up:
                              _swiglu_quad(st["c12"], c12, m, 0, 1)
                          else:
                              st["c12"] = c12
                      elif q == 6:      # W = U1 + P7; C22 = V + P7
                          w_ = tmp.tile([P, nb], bf16, tag="w", bufs=2)
                          nc.vector.tensor_add(w_[:], S["u1"][:], pt[:])
                          S["w"] = w_
                          c22 = tmp.tile([P, nb], bf16, tag="c22g", bufs=2)
                          nc.vector.tensor_add(c22[:], S["v"][:], pt[:])
                          if is_up:
                              _swiglu_quad(st["c22"], c22, m, 1, 1)
                          else:
                              st["c22"] = c22
                      elif q == 3:      # C21 = W + P4'  (B-combo negated)
                          c21 = tmp.tile([P, nb], bf16, tag="c21g", bufs=2)
                          nc.vector.tensor_add(c21[:], S["w"][:], pt[:])
                          if is_up:
                              _swiglu_quad(st["c21"], c21, m, 1, 0)
                          else:
                              st["c21"] = c21
                      else:             # q == 1: psA now holds C11
                          if not is_up:
                              sig = tmp.tile([P, nb], bf16, tag="sig", bufs=2)
                              nc.scalar.activation(sig[:], pt[:], Sig)
                              sg = tmp.tile([P, nb], bf16, tag="sg11", bufs=2)
                              nc.vector.tensor_mul(sg[:], sig[:], pt[:])
                              st["sg11"] = sg
                          else:
                              nc.vector.tensor_mul(
                                  interT[:, m * C: m * C + nb],
                                  st["sg11"][:], pt[:])

                  def position(wam, is_up, m):
                      S = {}
                      for q in ORDER:
                          do_product(q, wam, S, is_up, m)

                  def do_product_m0(q, wam, S, is_up):
                      """Position-0 variant: products in data-arrival order
                      (x quadrants land before the DVE-built T-combos), all
                      Winograd drains deferred to P6/P4 time."""
                      if q in (0, 1):
                          if q == 0:
                              S["psA"] = psS.tile([P, nb], f32, tag="a",
                                                  bufs=2, name="psA_m0")
                          pt = S["psA"]
                      else:
                          pt = psS.tile([P, nb], f32, tag=BTAG[q], bufs=1,
                                        name=f"ps_{BTAG[q]}_m0")
                      first = (q != 1)
                      for k in range(kt):
                          nc.tensor.matmul(
                              pt[:], wam[q][:, k * P:(k + 1) * P],
                              rhs_q(q, k),
                              start=(first and k == 0),
                              stop=((not first) and k == kt - 1),
                              skip_group_check=True)
                      if q == 0:
                          p1 = tmp.tile([P, nb], bf16, tag="p1", bufs=2)
                          nc.scalar.copy(p1[:], pt[:])
                          S["p1"] = p1
                      elif q in (4, 2, 6):
                          S[f"ps{q}"] = pt
                      elif q == 1:      # psA now holds C11
                          if not is_up:
                              sig = tmp.tile([P, nb], bf16, tag="sig", bufs=2)
                              nc.scalar.activation(sig[:], pt[:], Sig)
                              sg = tmp.tile([P, nb], bf16, tag="sg11", bufs=2)
                              nc.vector.tensor_mul(sg[:], sig[:], pt[:])
                              st["sg11"] = sg
                          else:
                              nc.vector.tensor_mul(
                                  interT[:, 0:nb], st["sg11"][:], pt[:])
                      elif q == 5:      # P6: bunched drains
                          u1 = tmp.tile([P, nb], bf16, tag="u1", bufs=2)
                          nc.vector.tensor_add(u1[:], S["p1"][:], pt[:])
                          v = tmp.tile([P, nb], bf16, tag="v", bufs=2)
                          nc.vector.tensor_add(v[:], u1[:], S["ps4"][:])
                          w_ = tmp.tile([P, nb], bf16, tag="w", bufs=2)
                          nc.vector.tensor_add(w_[:], u1[:], S["ps6"][:])
                          S["w"] = w_
                          c12 = tmp.tile([P, nb], bf16, tag="c12g", bufs=2)
                          nc.vector.tensor_add(c12[:], v[:], S["ps2"][:])
                          c22 = tmp.tile([P, nb], bf16, tag="c22g", bufs=2)
                          nc.vector.tensor_add(c22[:], v[:], S["ps6"][:])
                          if is_up:
                              _swiglu_quad(st["c12"], c12, 0, 0, 1)
                              _swiglu_quad(st["c22"], c22, 0, 1, 1)
                          else:
                              st["c12"], st["c22"] = c12, c22
                      elif q == 3:      # P4': C21 = W + P4'
                          c21 = tmp.tile([P, nb], bf16, tag="c21g", bufs=2)
                          nc.vector.tensor_add(c21[:], S["w"][:], pt[:])
                          if is_up:
                              _swiglu_quad(st["c21"], c21, 0, 1, 0)
                          else:
                              st["c21"] = c21

                  def _swiglu_quad(cg, cu, m, rhalf, chalf):
                      sig = tmp.tile([P, nb], bf16, tag="sig", bufs=2)
                      nc.scalar.activation(sig[:], cg[:], Sig)
                      sgq = tmp.tile([P, nb], bf16, tag="sgq", bufs=2)
                      nc.vector.tensor_mul(sgq[:], sig[:], cg[:])
                      i = (mt if rhalf else 0) + m
                      off = nb if chalf else 0
                      nc.vector.tensor_mul(
                          interT[:, i * C + off: i * C + off + nb],
                          sgq[:], cu[:])

                  for m in range(mt):
                      if m == 0:
                          Sg, Su = {}, {}
                          for q in ORDER_M0:
                              do_product_m0(q, wam_g, Sg, False)
                          for q in ORDER_M0:
                              do_product_m0(q, wam_u, Su, True)
                          wam_g = load_wa(wgs, 1, 0)
                          continue
                      # prefetch: next half-position's stationary operands
                      wam_u = load_wa(wus, m, 1)
                      position(wam_g, False, m)
                      if m + 1 < mt:
                          wam_g_next = load_wa(wgs, m + 1, 0)
                      if m == mt - 3:
                          # prefetch the first phase-2 stationary tiles
                          # (only w2r.bufs of them — more would block the
                          # ring engines on slot reuse until phase 2 runs)
                          for d in range(4):
                              eng = nc.sync if d % 2 == 0 else nc.scalar
                              stat0.append(stat_load(QORD2[0], d, eng))
                      position(wam_u, True, m)
                      if m + 1 < mt:
                          wam_g = wam_g_next

            # ------- Phase 2: down GEMM, Strassen-Winograd q-outer -------
            # outT = wdT @ interT.  Stationary = host-combined wd tiles;
            # moving = interT quadrant views + 4 on-device combos, each
            # built by the DVE one product-pass ahead.  Product passes
            # accumulate into per-quadrant bf16 accumulators (ping-pong
            # generations, so no in-place aliasing).
            USES = {0: ("c11", "c12", "c21", "c22"), 1: ("c11",),
                    2: ("c12",), 4: ("c12", "c22"), 6: ("c21", "c22"),
                    5: ("c12", "c21", "c22"), 3: ("c21",)}
            LAST = {"c11": 1, "c12": 5, "c22": 5, "c21": 3}
            OUT_OFF = {"c11": (0, 0), "c12": (0, 1),
                       "c21": (1, 0), "c22": (1, 1)}
            # moving operand per product: interT quadrant view or combo kind
            XOPV = {0: (0, 0), 1: (1, 0), 2: (1, 1),
                    4: "T1", 6: "T3", 5: "T2", 3: "T4n"}
            BUILD = {2: "T1", 3: "T3", 4: "T2", 5: "T4n"}

            def bv(rhalf, chalf, k):
                i = (kt2 if rhalf else 0) + k
                off = nb if chalf else 0
                return interT[:, i * C + off: i * C + off + nb]

            with tc.tile_pool(name="p2", bufs=2) as p2, \
               tc.tile_pool(name="ps2", bufs=4, space="PSUM") as ps2:
                tq = {}

                def build_T(dst, k, kind):
                    if kind == "T1":     # B12 - B11
                        nc.vector.tensor_sub(dst, bv(0, 1, k), bv(0, 0, k))
                    elif kind == "T3":   # B22 - B12
                        nc.vector.tensor_sub(dst, bv(1, 1, k), bv(0, 1, k))
                    elif kind == "T2":   # B22 - B12 + B11 = T3 + B11
                        nc.vector.tensor_add(
                            dst, tq["T3"][:, k * nb:(k + 1) * nb], bv(0, 0, k))
                    elif kind == "T4n":  # B21 - T2
                        nc.vector.tensor_sub(
                            dst, bv(1, 0, k), tq["T2"][:, k * nb:(k + 1) * nb])

                genct = {"c11": 0, "c12": 0, "c21": 0, "c22": 0}
                cbuf = {}
                statq = {0: stat0}
                for qi, q in enumerate(QORD2):
                    # finish prefetching this pass's stationary tiles (the
                    # phase-1 tail only seeded w2r.bufs of them), then issue
                    # the next pass's
                    if qi == 0:
                        for m in range(len(stat0), mtH):
                            eng = nc.sync if m % 2 == 0 else nc.scalar
                            stat0.append(stat_load(q, m, eng))
                    if qi + 1 < len(QORD2):
                        nq = QORD2[qi + 1]
                        statq[qi + 1] = [
                            stat_load(nq, m,
                                      nc.sync if m % 2 == 0 else nc.scalar)
                            for m in range(mtH)]
                    if qi in BUILD:
                        kind = BUILD[qi]
                        dst = p2.tile([P, kt2 * nb], bf16, tag="tq")
                        for k in range(kt2):
                            build_T(dst[:, k * nb:(k + 1) * nb], k, kind)
                        tq[kind] = dst
                    newgen = {cq: p2.tile([P, mtH * nb], bf16, tag=cq,
                                          name=f"acc_{cq}_{qi}")
                              for cq in USES[q]}
                    mov = XOPV[q]
                    for m in range(mtH):
                        ps = ps2.tile([P, nb], f32, tag="o")
                        stt = statq[qi][m]
                        for k in range(kt2):
                            rhs = (bv(mov[0], mov[1], k)
                                   if isinstance(mov, tuple)
                                   else tq[mov][:, k * nb:(k + 1) * nb])
                            nc.tensor.matmul(ps[:], stt[:, k * P:(k + 1) * P],
                                             rhs, start=(k == 0),
                                             stop=(k == kt2 - 1))
                        for cq in USES[q]:
                            dstm = newgen[cq][:, m * nb:(m + 1) * nb]
                            if genct[cq] == 0:
                                nc.scalar.copy(dstm, ps[:])
                            else:
                                nc.vector.tensor_add(
                                    dstm, cbuf[cq][:, m * nb:(m + 1) * nb],
                                    ps[:])
                            if LAST[cq] == q:
                                # stream each finished quadrant slice out as
                                # soon as its final add lands (keeps the
                                # kernel tail to one 128 KB transfer)
                                r, c_ = OUT_OFF[cq]
                                nc.sync.dma_start(
                                    out[(r * mtH + m) * P:
                                        (r * mtH + m + 1) * P,
                                        c_ * nb:(c_ + 1) * nb], dstm)
                    for cq in USES[q]:
                        cbuf[cq] = newgen[cq]
                        genct[cq] += 1

    nc.compile()
    return nc


def _get_nc(C, Hd, Id):
    key = (C, Hd, Id, _use_strassen(C, Hd, Id))
    if key not in _NC_CACHE:
        if _use_strassen(C, Hd, Id):
            _NC_CACHE[key] = _build_strassen(C, Hd, Id)
        else:
            _NC_CACHE[key] = _build(C, Hd, Id)
    return _NC_CACHE[key]


def _tile_x(xe, Hd, C):
    """[C, Hd] fp32 -> [P, KT*C] bf16 with xT[p, k*C+c] = x[c, k*P+p]."""
    KT = Hd // P
    t = xe.T.reshape(KT, P, C).transpose(1, 0, 2).reshape(P, KT * C)
    return np.ascontiguousarray(t).astype(ml_dtypes.bfloat16)


def _tile_w1(w, Hd, Id):
    """[Hd, Id] fp32 -> [IP*P, KT*W2] bf16 (pair-tiled gate/up layout)."""
    KT = Hd // P
    IP = Id // (2 * P)
    W2 = 2 * P
    t = w.reshape(KT, P, IP, W2).transpose(2, 1, 0, 3).reshape(IP * P, KT * W2)
    return np.ascontiguousarray(t).astype(ml_dtypes.bfloat16)


def _tile_wd(w, Id, Hd, nb=NB):
    """[Id, Hd] fp32 -> [HB*P, IT*nb] bf16 (h-block-tiled down layout)."""
    IT = Id // P
    HB = Hd // nb
    t = w.reshape(IT, P, HB, nb).transpose(2, 1, 0, 3).reshape(HB * P, IT * nb)
    return np.ascontiguousarray(t).astype(ml_dtypes.bfloat16)


def _wops(w, Kh, Mh):
    """The 7 Winograd-Strassen stationary-side operand combos, in the
    weight's own [K, M] orientation, ordered P1..P7."""
    Q11 = w[:Kh, :Mh]; Q12 = w[:Kh, Mh:]
    Q21 = w[Kh:, :Mh]; Q22 = w[Kh:, Mh:]
    return [Q11, Q21, Q11 + Q21 - Q12 - Q22, Q22, Q12 + Q22,
            Q12 + Q22 - Q11, Q11 - Q12]


def _tile_wq(ops, ktX, mtX):
    a = np.stack(ops)  # [7, K, M]
    a = (a.reshape(7, ktX, P, mtX, P).transpose(0, 3, 2, 1, 4)
         .reshape(7 * mtX * P, ktX * P))
    return np.ascontiguousarray(a).astype(ml_dtypes.bfloat16)


def _prepare_strassen_expert(xe, g, u, dn, Hd, Id, C):
    """Host-side Strassen-Winograd operand combos + tiling for one expert.
    All in fp32, cast to bf16 at the end.  The P4 x-combo is negated so the
    device only ever adds product tiles."""
    bf = ml_dtypes.bfloat16
    Kh, Mh, Nh = Hd // 2, Id // 2, C // 2
    kt, mt = Kh // P, Mh // P

    return {
        "xT": _tile_x(xe, Hd, C),
        "wgs": _tile_wq(_wops(g, Kh, Mh), kt, mt),
        "wus": _tile_wq(_wops(u, Kh, Mh), kt, mt),
        "wds": _tile_wq(_wops(dn, Id // 2, Hd // 2), Id // (2 * P),
                        Hd // (2 * P)),
    }


def _prepare(inputs):
    """Host-side dispatch: split tokens by expert, pad to capacity, cast to
    bf16, and pre-tile everything into the device SBUF layouts so all DMAs
    are contiguous."""
    x = np.asarray(inputs["permuted_local_hidden_states"], dtype=np.float32)
    tpe = np.asarray(inputs["tokens_per_expert"], dtype=np.int64)
    gate = np.asarray(inputs["gate_proj"], dtype=np.float32)
    up = np.asarray(inputs["up_proj"], dtype=np.float32)
    down = np.asarray(inputs["down_proj"], dtype=np.float32)

    Ee, Hd, Id = gate.shape
    Tt = x.shape[0]
    assert Ee == E, f"expected {E} experts, got {Ee}"
    counts = [int(c) for c in tpe]
    starts = [0]
    for c in counts:
        starts.append(starts[-1] + c)
    cmax = max(max(counts), 1)
    # round capacity to a multiple of NB so TT = C//NB tiles exactly
    C = max(C_DEFAULT, ((cmax + NB - 1) // NB) * NB)

    use_st = _use_strassen(C, Hd, Id)
    in_maps = []
    for e in range(Ee):
        s, cnt = starts[e], counts[e]
        if cnt == C:
            xe = x[s:s + cnt]
        else:
            xe = np.zeros((C, Hd), np.float32)
            xe[:cnt] = x[s:s + cnt]
        if use_st:
            in_maps.append(
                _prepare_strassen_expert(xe, gate[e], up[e], down[e],
                                         Hd, Id, C))
        else:
            in_maps.append({
                "xT": _tile_x(xe, Hd, C),
                "wg": _tile_w1(gate[e], Hd, Id),
                "wu": _tile_w1(up[e], Hd, Id),
                "wd": _tile_wd(down[e], Id, Hd),
            })
    meta = (Tt, Hd, starts, counts, C, use_st)
    return in_maps, meta


def _postprocess(results, meta):
    Tt, Hd, starts, counts, _C, use_st = meta
    outf = np.zeros((Tt, Hd), np.float32)
    for e in range(len(counts)):
        s, cnt = starts[e], counts[e]
        if cnt > 0:
            arr = np.asarray(results[e]["out"])
            if use_st:
                arr = arr.T  # strassen build emits outT [Hd, C]
            outf[s:s + cnt] = arr[:cnt].astype(np.float32)
    return outf


def kernel(**inputs):
    from concourse.bass_utils import run_bass_kernel_spmd
    in_maps, meta = _prepare(inputs)
    nc = _get_nc(meta[4], meta[1], np.asarray(inputs["gate_proj"]).shape[2])
    res = run_bass_kernel_spmd(nc, in_maps, list(range(E)))
    return _postprocess(res.results, meta)
